# revision 7
# baseline (speedup 1.0000x reference)
"""Multi-head attention Bass/Tile kernel for Trainium2, 8-core SPMD.

Problem: B=2, T=4096, D=512, H=8 heads (hd=64). Outputs (out, attn):
  out  [B, T, D]     = softmax(QK^T/8) V Wo^T + bo
  attn [B, H, T, T]  = softmax probabilities (the dominant 1.07 GB output)

Sharding: 16 (batch, head) jobs -> 8 cores, 2 heads of one batch per core.
Each core:
  - projects q, k (layout [2*64 head-dims across partitions, T]) and
    v (natural layout [s partitions, 2*64 head cols])
  - normal pass: scores[t,s] per 128-row t-block -> exp (ACT, with
    per-partition row-sum accumulation) -> multiply by 1/rowsum -> DMA to
    the attn output
  - transposed pass: scores[s,t] recomputed -> exp -> attn^T @ v matmuls
    accumulate z^T[e, t] unnormalized in PSUM (t in quarters)
  - output projection z @ Wo_cols^T with the 1/rowsum folded in as a
    per-partition scalar multiply; partial out summed across cores on host.
"""

import numpy as np

B, H, T, D, HD, NJ, NCORES = 2, 8, 4096, 512, 64, 2, 8


def build_mha(tc, outs, ins, T=T):
    import concourse.mybir as mybir

    nc = tc.nc
    F32 = mybir.dt.float32
    AF = mybir.ActivationFunctionType
    ALU = mybir.AluOpType
    AX = mybir.AxisListType

    xqT, xkT, xvT = ins["xqT"], ins["xkT"], ins["xvT"]
    wqT, wkT, wvT, woT = ins["wqT"], ins["wkT"], ins["wvT"], ins["woT"]
    attn2, outp = outs["attn2"], outs["outp"]

    DC = D // 128            # d-chunks (4)
    TB = T // 128            # t/s 128-blocks
    NPW = min(1024, T)       # normal-pass psum width
    NPC = T // NPW           # chunks per t-block row
    TS = NPW                 # projection t-slice
    NTS = T // TS
    ZQW = min(512, T)        # z^T accumulator width (one PSUM bank per head)
    NQ = T // ZQW
    SCALE = 1.0 / 8.0        # 1/sqrt(hd)

    xqT_v = xqT.rearrange("(c p) t -> p c t", p=128)
    xkT_v = xkT.rearrange("(c p) t -> p c t", p=128)
    xvT_v = xvT.rearrange("(c p) t -> p c t", p=128)

    with (
        tc.tile_pool(name="xin", bufs=2) as xin_p,
        tc.tile_pool(name="wgt", bufs=1) as w_p,
        tc.tile_pool(name="per", bufs=1) as per_p,
        tc.tile_pool(name="exp", bufs=3) as exp_p,
        tc.tile_pool(name="expt", bufs=3) as expt_p,
        tc.tile_pool(name="osb", bufs=3) as osb_p,
        tc.tile_pool(name="sm", bufs=6) as sm_p,
        tc.tile_pool(name="np_ps", bufs=2, space="PSUM") as np_ps,
        tc.tile_pool(name="st_ps", bufs=2, space="PSUM") as st_ps,
        tc.tile_pool(name="zt_ps", bufs=1, space="PSUM") as zt_ps,
    ):
        # ---- weights ----
        wq_sb = w_p.tile([128, DC, 128], F32, tag="wq")
        nc.sync.dma_start(out=wq_sb[:], in_=wqT.rearrange("(c p) j -> p c j", p=128))
        wk_sb = w_p.tile([128, DC, 128], F32, tag="wk")
        nc.sync.dma_start(out=wk_sb[:], in_=wkT.rearrange("(c p) j -> p c j", p=128))
        wv_sb = w_p.tile([128, DC, 128], F32, tag="wv")
        nc.sync.dma_start(out=wv_sb[:], in_=wvT.rearrange("(c p) j -> p c j", p=128))
        wo_sb = w_p.tile([64, NJ, D], F32, tag="wo")
        nc.sync.dma_start(
            out=wo_sb[:], in_=woT.rearrange("(j p) o -> p j o", p=64)
        )

        # ---- persistent tiles ----
        qT2 = per_p.tile([128, T], F32, tag="qT2")   # [2 heads x 64 hd, t]
        kT2 = per_p.tile([128, T], F32, tag="kT2")
        v2 = per_p.tile([128, T], F32, tag="v2")     # [s%128, sblock*128 + e]
        zT2 = per_p.tile([64, NJ, T], F32, tag="zT2")  # [e, head, t] unnormalized
        recip = per_p.tile([128, NJ * TB], F32, tag="recip")

        # ---- projections ----
        for ts in range(NTS):
            tsl = slice(ts * TS, (ts + 1) * TS)
            xq_t = xin_p.tile([128, DC, TS], F32, tag="xin")
            nc.sync.dma_start(out=xq_t[:], in_=xqT_v[:, :, tsl])
            xk_t = xin_p.tile([128, DC, TS], F32, tag="xin")
            nc.sync.dma_start(out=xk_t[:], in_=xkT_v[:, :, tsl])
            xv_t = xin_p.tile([128, DC, TS], F32, tag="xin")
            nc.sync.dma_start(out=xv_t[:], in_=xvT_v[:, :, tsl])

            for w_sb, x_t, dst in ((wq_sb, xq_t, qT2), (wk_sb, xk_t, kT2)):
                ps = np_ps.tile([128, NPW], F32, tag="np")
                for d in range(DC):
                    for c in range(TS // 512):
                        nc.tensor.matmul(
                            ps[:, c * 512 : (c + 1) * 512],
                            w_sb[:, d, :],
                            x_t[:, d, c * 512 : (c + 1) * 512],
                            start=(d == 0),
                            stop=(d == DC - 1),
                        )
                nc.vector.tensor_copy(dst[:, tsl], ps[:])

            # v in natural layout: out[s, e] per 128-s-block
            # (one bank-owning accumulation group per tile: start= clears the
            # whole bank's has_written bits, so regions can't share a bank)
            for sb in range(TS // 128):
                ps = st_ps.tile([128, 512], F32, tag="st")
                for d in range(DC):
                    nc.tensor.matmul(
                        ps[:, 0:128],
                        xv_t[:, d, sb * 128 : (sb + 1) * 128],
                        wv_sb[:, d, :],
                        start=(d == 0),
                        stop=(d == DC - 1),
                    )
                base = ts * TS + sb * 128
                nc.vector.tensor_copy(v2[:, base : base + 128], ps[:, 0:128])

        # ---- normal pass: attn rows out ----
        for tb in range(TB):
            for j in range(NJ):
                jp = slice(64 * j, 64 * j + 64)
                et = exp_p.tile([128, T], F32, tag="exp")
                rs = sm_p.tile([128, NPC], F32, tag="rs")
                for c in range(NPC):
                    ps = np_ps.tile([128, NPW], F32, tag="np")
                    for cc in range(NPW // 512):
                        sl = slice(c * NPW + cc * 512, c * NPW + (cc + 1) * 512)
                        nc.tensor.matmul(
                            ps[:, cc * 512 : (cc + 1) * 512],
                            qT2[jp, tb * 128 : (tb + 1) * 128],
                            kT2[jp, sl],
                            start=True,
                            stop=True,
                        )
                    nc.scalar.activation(
                        et[:, c * NPW : (c + 1) * NPW],
                        ps[:],
                        AF.Exp,
                        scale=SCALE,
                        accum_out=rs[:, c : c + 1],
                    )
                rsum = sm_p.tile([128, 1], F32, tag="rsum")
                nc.vector.tensor_reduce(rsum[:], rs[:], axis=AX.X, op=ALU.add)
                ridx = j * TB + tb
                nc.vector.reciprocal(recip[:, ridx : ridx + 1], rsum[:])
                nc.vector.tensor_scalar_mul(et[:], et[:], recip[:, ridx : ridx + 1])
                nc.sync.dma_start(
                    out=attn2[j, tb * 128 : (tb + 1) * 128, :], in_=et[:]
                )

        # ---- transposed pass + attn^T v ----
        # each head's z^T accumulator owns its own PSUM bank for the whole
        # s-loop (start= clears has_written bank-wide)
        for q in range(NQ):
            tsl = slice(q * ZQW, (q + 1) * ZQW)
            zqs = [
                zt_ps.tile([64, ZQW], F32, tag=f"zt{j}", name=f"zq{j}")
                for j in range(NJ)
            ]
            for s in range(TB):
                for j in range(NJ):
                    jp = slice(64 * j, 64 * j + 64)
                    ps = st_ps.tile([128, ZQW], F32, tag="st")
                    nc.tensor.matmul(
                        ps[:, :],
                        kT2[jp, s * 128 : (s + 1) * 128],
                        qT2[jp, tsl],
                        start=True,
                        stop=True,
                    )
                    ex = expt_p.tile([128, ZQW], F32, tag="expt")
                    nc.scalar.activation(ex[:], ps[:], AF.Exp, scale=SCALE)
                    nc.tensor.matmul(
                        zqs[j][:, :],
                        v2[:, s * 128 + 64 * j : s * 128 + 64 * j + 64],
                        ex[:],
                        start=(s == 0),
                        stop=(s == TB - 1),
                    )
            for j in range(NJ):
                nc.vector.tensor_copy(zT2[:, j, tsl], zqs[j][:, :])

        # ---- output projection (recip folded in per head) ----
        for tb in range(TB):
            po0 = st_ps.tile([128, 512], F32, tag="st")
            nc.tensor.matmul(
                po0[:, 0:D], zT2[:, 0, tb * 128 : (tb + 1) * 128], wo_sb[:, 0, :],
                start=True, stop=True,
            )
            po1 = st_ps.tile([128, 512], F32, tag="st")
            nc.tensor.matmul(
                po1[:, 0:D], zT2[:, 1, tb * 128 : (tb + 1) * 128], wo_sb[:, 1, :],
                start=True, stop=True,
            )
            tmp = osb_p.tile([128, D], F32, tag="tmp")
            nc.vector.tensor_scalar_mul(tmp[:], po0[:, 0:D], recip[:, tb : tb + 1])
            ot = osb_p.tile([128, D], F32, tag="ot")
            nc.vector.scalar_tensor_tensor(
                ot[:], po1[:, 0:D], recip[:, TB + tb : TB + tb + 1], tmp[:],
                op0=ALU.mult, op1=ALU.add,
            )
            nc.sync.dma_start(out=outp[tb * 128 : (tb + 1) * 128, :], in_=ot[:])


def _build_program(T=T):
    import concourse.mybir as mybir
    import concourse.tile as tile
    from concourse import bacc

    F32 = mybir.dt.float32
    nc = bacc.Bacc(
        "TRN2", target_bir_lowering=False, debug=False, num_devices=NCORES
    )
    ins = {
        "xqT": nc.dram_tensor("xqT", [D, T], F32, kind="ExternalInput").ap(),
        "xkT": nc.dram_tensor("xkT", [D, T], F32, kind="ExternalInput").ap(),
        "xvT": nc.dram_tensor("xvT", [D, T], F32, kind="ExternalInput").ap(),
        "wqT": nc.dram_tensor("wqT", [D, 128], F32, kind="ExternalInput").ap(),
        "wkT": nc.dram_tensor("wkT", [D, 128], F32, kind="ExternalInput").ap(),
        "wvT": nc.dram_tensor("wvT", [D, 128], F32, kind="ExternalInput").ap(),
        "woT": nc.dram_tensor("woT", [128, D], F32, kind="ExternalInput").ap(),
    }
    outs = {
        "attn2": nc.dram_tensor("attn2", [NJ, T, T], F32, kind="ExternalOutput").ap(),
        "outp": nc.dram_tensor("outp", [T, D], F32, kind="ExternalOutput").ap(),
    }
    with tile.TileContext(nc) as tc:
        build_mha(tc, outs, ins, T=T)
    nc.compile()
    return nc


_PROGRAM_CACHE = {}


def run_cores(in_maps, T=T):
    from concourse.bass_utils import run_bass_kernel_spmd

    if T not in _PROGRAM_CACHE:
        _PROGRAM_CACHE[T] = _build_program(T)
    nc = _PROGRAM_CACHE[T]
    return run_bass_kernel_spmd(nc, in_maps, list(range(NCORES))).results


def make_in_maps(queries, keys, values, Wq, Wk, Wv, Wo):
    f32 = np.float32
    in_maps = []
    for c in range(NCORES):
        b = c // 4
        h0 = 2 * (c % 4)
        rows = slice(h0 * HD, (h0 + 2) * HD)
        in_maps.append(
            {
                "xqT": np.ascontiguousarray(np.asarray(queries)[b].T, f32),
                "xkT": np.ascontiguousarray(np.asarray(keys)[b].T, f32),
                "xvT": np.ascontiguousarray(np.asarray(values)[b].T, f32),
                "wqT": np.ascontiguousarray(np.asarray(Wq)[rows].T, f32),
                "wkT": np.ascontiguousarray(np.asarray(Wk)[rows].T, f32),
                "wvT": np.ascontiguousarray(np.asarray(Wv)[rows].T, f32),
                "woT": np.ascontiguousarray(np.asarray(Wo)[:, rows].T, f32),
            }
        )
    return in_maps


def kernel(queries, keys, values, Wq, Wk, Wv, Wo, bo):
    in_maps = make_in_maps(queries, keys, values, Wq, Wk, Wv, Wo)
    res = run_cores(in_maps)
    bo = np.asarray(bo, np.float32)
    out = np.empty((B, T, D), np.float32)
    attn = np.empty((B, H, T, T), np.float32)
    for b in range(B):
        acc = None
        for c in range(4 * b, 4 * b + 4):
            h0 = 2 * (c % 4)
            attn[b, h0 : h0 + 2] = res[c]["attn2"]
            acc = res[c]["outp"] if acc is None else acc + res[c]["outp"]
        out[b] = acc + bo[None, :]
    return out, attn


# revision 9
# speedup vs baseline: 403.3702x; 403.3702x over previous
"""Multi-head attention Bass/Tile kernel for Trainium2, 8-core SPMD.

Problem: B=2, T=4096, D=512, H=8 heads (hd=64). Outputs (out, attn):
  out  [B, T, D]     = softmax(QK^T/8) V Wo^T + bo
  attn [B, H, T, T]  = softmax probabilities (the dominant 1.07 GB output)

Sharding: 16 (batch, head) jobs -> 8 cores, 2 heads of one batch per core.
Each core:
  - projects q, k (layout [2*64 head-dims across partitions, T]) and
    v (natural layout [s partitions, 2*64 head cols])
  - normal pass: scores[t,s] per 128-row t-block -> exp (ACT, with
    per-partition row-sum accumulation) -> multiply by 1/rowsum -> DMA to
    the attn output
  - transposed pass: scores[s,t] recomputed -> exp -> attn^T @ v matmuls
    accumulate z^T[e, t] unnormalized in PSUM (t in quarters)
  - output projection z @ Wo_cols^T with the 1/rowsum folded in as a
    per-partition scalar multiply; partial out summed across cores on host.
"""

import numpy as np

B, H, T, D, HD, NJ, NCORES = 2, 8, 4096, 512, 64, 2, 8


def build_mha(tc, outs, ins, T=T):
    import concourse.mybir as mybir

    nc = tc.nc
    F32 = mybir.dt.float32
    F32R = mybir.dt.float32r  # PE processes f32r at ~4x the rate of f32

    def r(ap):
        return ap.bitcast(F32R)

    AF = mybir.ActivationFunctionType
    ALU = mybir.AluOpType
    AX = mybir.AxisListType

    xqT, xkT, xvT = ins["xqT"], ins["xkT"], ins["xvT"]
    wqT, wkT, wvT, woT = ins["wqT"], ins["wkT"], ins["wvT"], ins["woT"]
    attn2, outp = outs["attn2"], outs["outp"]

    DC = D // 128            # d-chunks (4)
    TB = T // 128            # t/s 128-blocks
    NPW = min(1024, T)       # normal-pass psum width
    NPC = T // NPW           # chunks per t-block row
    TS = NPW                 # projection t-slice
    NTS = T // TS
    ZQW = min(512, T)        # z^T accumulator width (one PSUM bank per head)
    NQ = T // ZQW
    SCALE = 1.0 / 8.0        # 1/sqrt(hd)

    xqT_v = xqT.rearrange("(c p) t -> p c t", p=128)
    xkT_v = xkT.rearrange("(c p) t -> p c t", p=128)
    xvT_v = xvT.rearrange("(c p) t -> p c t", p=128)

    with (
        tc.tile_pool(name="xin", bufs=2) as xin_p,
        tc.tile_pool(name="wgt", bufs=1) as w_p,
        tc.tile_pool(name="per", bufs=1) as per_p,
        tc.tile_pool(name="exp", bufs=3) as exp_p,
        tc.tile_pool(name="expt", bufs=3) as expt_p,
        tc.tile_pool(name="osb", bufs=3) as osb_p,
        tc.tile_pool(name="sm", bufs=6) as sm_p,
        tc.tile_pool(name="np_ps", bufs=2, space="PSUM") as np_ps,
        tc.tile_pool(name="st_ps", bufs=2, space="PSUM") as st_ps,
        tc.tile_pool(name="zt_ps", bufs=1, space="PSUM") as zt_ps,
    ):
        # ---- weights ----
        wq_sb = w_p.tile([128, DC, 128], F32, tag="wq")
        nc.sync.dma_start(out=wq_sb[:], in_=wqT.rearrange("(c p) j -> p c j", p=128))
        wk_sb = w_p.tile([128, DC, 128], F32, tag="wk")
        nc.sync.dma_start(out=wk_sb[:], in_=wkT.rearrange("(c p) j -> p c j", p=128))
        wv_sb = w_p.tile([128, DC, 128], F32, tag="wv")
        nc.sync.dma_start(out=wv_sb[:], in_=wvT.rearrange("(c p) j -> p c j", p=128))
        wo_sb = w_p.tile([64, NJ, D], F32, tag="wo")
        nc.sync.dma_start(
            out=wo_sb[:], in_=woT.rearrange("(j p) o -> p j o", p=64)
        )

        # ---- persistent tiles ----
        qT2 = per_p.tile([128, T], F32, tag="qT2")   # [2 heads x 64 hd, t]
        kT2 = per_p.tile([128, T], F32, tag="kT2")
        v2 = per_p.tile([128, T], F32, tag="v2")     # [s%128, sblock*128 + e]
        zT2 = per_p.tile([64, NJ, T], F32, tag="zT2")  # [e, head, t] unnormalized
        recip = per_p.tile([128, NJ * TB], F32, tag="recip")

        # ---- projections ----
        for ts in range(NTS):
            tsl = slice(ts * TS, (ts + 1) * TS)
            xq_t = xin_p.tile([128, DC, TS], F32, tag="xin")
            nc.sync.dma_start(out=xq_t[:], in_=xqT_v[:, :, tsl])
            xk_t = xin_p.tile([128, DC, TS], F32, tag="xin")
            nc.sync.dma_start(out=xk_t[:], in_=xkT_v[:, :, tsl])
            xv_t = xin_p.tile([128, DC, TS], F32, tag="xin")
            nc.sync.dma_start(out=xv_t[:], in_=xvT_v[:, :, tsl])

            for w_sb, x_t, dst in ((wq_sb, xq_t, qT2), (wk_sb, xk_t, kT2)):
                ps = np_ps.tile([128, NPW], F32, tag="np")
                for d in range(DC):
                    for c in range(TS // 512):
                        nc.tensor.matmul(
                            ps[:, c * 512 : (c + 1) * 512],
                            r(w_sb[:, d, :]),
                            r(x_t[:, d, c * 512 : (c + 1) * 512]),
                            start=(d == 0),
                            stop=(d == DC - 1),
                        )
                nc.vector.tensor_copy(dst[:, tsl], ps[:])

            # v in natural layout: out[s, e] per 128-s-block
            # (one bank-owning accumulation group per tile: start= clears the
            # whole bank's has_written bits, so regions can't share a bank)
            for sb in range(TS // 128):
                ps = st_ps.tile([128, 512], F32, tag="st")
                for d in range(DC):
                    nc.tensor.matmul(
                        ps[:, 0:128],
                        r(xv_t[:, d, sb * 128 : (sb + 1) * 128]),
                        r(wv_sb[:, d, :]),
                        start=(d == 0),
                        stop=(d == DC - 1),
                    )
                base = ts * TS + sb * 128
                nc.vector.tensor_copy(v2[:, base : base + 128], ps[:, 0:128])

        # ---- normal pass: attn rows out ----
        for tb in range(TB):
            for j in range(NJ):
                jp = slice(64 * j, 64 * j + 64)
                et = exp_p.tile([128, T], F32, tag="exp")
                rs = sm_p.tile([128, NPC], F32, tag="rs")
                for c in range(NPC):
                    ps = np_ps.tile([128, NPW], F32, tag="np")
                    for cc in range(NPW // 512):
                        sl = slice(c * NPW + cc * 512, c * NPW + (cc + 1) * 512)
                        nc.tensor.matmul(
                            ps[:, cc * 512 : (cc + 1) * 512],
                            r(qT2[jp, tb * 128 : (tb + 1) * 128]),
                            r(kT2[jp, sl]),
                            start=True,
                            stop=True,
                        )
                    nc.scalar.activation(
                        et[:, c * NPW : (c + 1) * NPW],
                        ps[:],
                        AF.Exp,
                        scale=SCALE,
                        accum_out=rs[:, c : c + 1],
                    )
                rsum = sm_p.tile([128, 1], F32, tag="rsum")
                nc.vector.tensor_reduce(rsum[:], rs[:], axis=AX.X, op=ALU.add)
                ridx = j * TB + tb
                nc.vector.reciprocal(recip[:, ridx : ridx + 1], rsum[:])
                nc.vector.tensor_scalar_mul(et[:], et[:], recip[:, ridx : ridx + 1])
                nc.sync.dma_start(
                    out=attn2[j, tb * 128 : (tb + 1) * 128, :], in_=et[:]
                )

        # ---- transposed pass + attn^T v ----
        # each head's z^T accumulator owns its own PSUM bank for the whole
        # s-loop (start= clears has_written bank-wide)
        for q in range(NQ):
            tsl = slice(q * ZQW, (q + 1) * ZQW)
            zqs = [
                zt_ps.tile([64, ZQW], F32, tag=f"zt{j}", name=f"zq{j}")
                for j in range(NJ)
            ]
            for s in range(TB):
                for j in range(NJ):
                    jp = slice(64 * j, 64 * j + 64)
                    ps = st_ps.tile([128, ZQW], F32, tag="st")
                    nc.tensor.matmul(
                        ps[:, :],
                        r(kT2[jp, s * 128 : (s + 1) * 128]),
                        r(qT2[jp, tsl]),
                        start=True,
                        stop=True,
                    )
                    ex = expt_p.tile([128, ZQW], F32, tag="expt")
                    nc.scalar.activation(ex[:], ps[:], AF.Exp, scale=SCALE)
                    nc.tensor.matmul(
                        zqs[j][:, :],
                        r(v2[:, s * 128 + 64 * j : s * 128 + 64 * j + 64]),
                        r(ex[:]),
                        start=(s == 0),
                        stop=(s == TB - 1),
                    )
            for j in range(NJ):
                nc.vector.tensor_copy(zT2[:, j, tsl], zqs[j][:, :])

        # ---- output projection (recip folded in per head) ----
        for tb in range(TB):
            po0 = st_ps.tile([128, 512], F32, tag="st")
            nc.tensor.matmul(
                po0[:, 0:D], r(zT2[:, 0, tb * 128 : (tb + 1) * 128]), r(wo_sb[:, 0, :]),
                start=True, stop=True,
            )
            po1 = st_ps.tile([128, 512], F32, tag="st")
            nc.tensor.matmul(
                po1[:, 0:D], r(zT2[:, 1, tb * 128 : (tb + 1) * 128]), r(wo_sb[:, 1, :]),
                start=True, stop=True,
            )
            tmp = osb_p.tile([128, D], F32, tag="tmp")
            nc.vector.tensor_scalar_mul(tmp[:], po0[:, 0:D], recip[:, tb : tb + 1])
            ot = osb_p.tile([128, D], F32, tag="ot")
            nc.vector.scalar_tensor_tensor(
                ot[:], po1[:, 0:D], recip[:, TB + tb : TB + tb + 1], tmp[:],
                op0=ALU.mult, op1=ALU.add,
            )
            nc.sync.dma_start(out=outp[tb * 128 : (tb + 1) * 128, :], in_=ot[:])


def _build_program(T=T):
    import concourse.mybir as mybir
    import concourse.tile as tile
    from concourse import bacc

    F32 = mybir.dt.float32
    nc = bacc.Bacc(
        "TRN2", target_bir_lowering=False, debug=False, num_devices=NCORES
    )
    ins = {
        "xqT": nc.dram_tensor("xqT", [D, T], F32, kind="ExternalInput").ap(),
        "xkT": nc.dram_tensor("xkT", [D, T], F32, kind="ExternalInput").ap(),
        "xvT": nc.dram_tensor("xvT", [D, T], F32, kind="ExternalInput").ap(),
        "wqT": nc.dram_tensor("wqT", [D, 128], F32, kind="ExternalInput").ap(),
        "wkT": nc.dram_tensor("wkT", [D, 128], F32, kind="ExternalInput").ap(),
        "wvT": nc.dram_tensor("wvT", [D, 128], F32, kind="ExternalInput").ap(),
        "woT": nc.dram_tensor("woT", [128, D], F32, kind="ExternalInput").ap(),
    }
    outs = {
        "attn2": nc.dram_tensor("attn2", [NJ, T, T], F32, kind="ExternalOutput").ap(),
        "outp": nc.dram_tensor("outp", [T, D], F32, kind="ExternalOutput").ap(),
    }
    with tile.TileContext(nc) as tc:
        build_mha(tc, outs, ins, T=T)
    nc.compile()
    return nc


_PROGRAM_CACHE = {}


def run_cores(in_maps, T=T):
    from concourse.bass_utils import run_bass_kernel_spmd

    if T not in _PROGRAM_CACHE:
        _PROGRAM_CACHE[T] = _build_program(T)
    nc = _PROGRAM_CACHE[T]
    return run_bass_kernel_spmd(nc, in_maps, list(range(NCORES))).results


def make_in_maps(queries, keys, values, Wq, Wk, Wv, Wo):
    f32 = np.float32
    in_maps = []
    for c in range(NCORES):
        b = c // 4
        h0 = 2 * (c % 4)
        rows = slice(h0 * HD, (h0 + 2) * HD)
        in_maps.append(
            {
                "xqT": np.ascontiguousarray(np.asarray(queries)[b].T, f32),
                "xkT": np.ascontiguousarray(np.asarray(keys)[b].T, f32),
                "xvT": np.ascontiguousarray(np.asarray(values)[b].T, f32),
                "wqT": np.ascontiguousarray(np.asarray(Wq)[rows].T, f32),
                "wkT": np.ascontiguousarray(np.asarray(Wk)[rows].T, f32),
                "wvT": np.ascontiguousarray(np.asarray(Wv)[rows].T, f32),
                "woT": np.ascontiguousarray(np.asarray(Wo)[:, rows].T, f32),
            }
        )
    return in_maps


def kernel(queries, keys, values, Wq, Wk, Wv, Wo, bo):
    in_maps = make_in_maps(queries, keys, values, Wq, Wk, Wv, Wo)
    res = run_cores(in_maps)
    bo = np.asarray(bo, np.float32)
    out = np.empty((B, T, D), np.float32)
    attn = np.empty((B, H, T, T), np.float32)
    for b in range(B):
        acc = None
        for c in range(4 * b, 4 * b + 4):
            h0 = 2 * (c % 4)
            attn[b, h0 : h0 + 2] = res[c]["attn2"]
            acc = res[c]["outp"] if acc is None else acc + res[c]["outp"]
        out[b] = acc + bo[None, :]
    return out, attn


# revision 10
# speedup vs baseline: 6924.7922x; 17.1673x over previous
"""Multi-head attention Bass/Tile kernel for Trainium2, 8-core SPMD.

Problem: B=2, T=4096, D=512, H=8 heads (hd=64). Outputs (out, attn):
  out  [B, T, D]     = softmax(QK^T/8) V Wo^T + bo
  attn [B, H, T, T]  = softmax probabilities (the dominant 1.07 GB output)

Sharding: 16 (batch, head) jobs -> 8 cores, 2 heads of one batch per core.
Each core:
  - projects q, k (layout [2*64 head-dims across partitions, T]) and
    v (natural layout [s partitions, 2*64 head cols])
  - normal pass: scores[t,s] per 128-row t-block -> exp (ACT, with
    per-partition row-sum accumulation) -> multiply by 1/rowsum -> DMA to
    the attn output
  - transposed pass: scores[s,t] recomputed -> exp -> attn^T @ v matmuls
    accumulate z^T[e, t] unnormalized in PSUM (t in quarters)
  - output projection z @ Wo_cols^T with the 1/rowsum folded in as a
    per-partition scalar multiply; partial out summed across cores on host.
"""

import numpy as np

B, H, T, D, HD, NJ, NCORES = 2, 8, 4096, 512, 64, 2, 8


def build_mha(tc, outs, ins, T=T):
    import concourse.mybir as mybir

    nc = tc.nc
    F32 = mybir.dt.float32
    F32R = mybir.dt.float32r  # PE processes f32r at ~4x the rate of f32

    def r(ap):
        return ap.bitcast(F32R)

    AF = mybir.ActivationFunctionType
    ALU = mybir.AluOpType
    AX = mybir.AxisListType

    xqT, xkT, xvT = ins["xqT"], ins["xkT"], ins["xvT"]
    wqT, wkT, wvT, woT = ins["wqT"], ins["wkT"], ins["wvT"], ins["woT"]
    attn2, outp = outs["attn2"], outs["outp"]

    DC = D // 128            # d-chunks (4)
    TB = T // 128            # t/s 128-blocks
    NPW = min(1024, T)       # normal-pass psum width
    NPC = T // NPW           # chunks per t-block row
    TS = NPW                 # projection t-slice
    NTS = T // TS
    ZQW = min(512, T)        # z^T accumulator width (one PSUM bank per head)
    NQ = T // ZQW
    SCALE = 1.0 / 8.0        # 1/sqrt(hd)

    xqT_v = xqT.rearrange("(c p) t -> p c t", p=128)
    xkT_v = xkT.rearrange("(c p) t -> p c t", p=128)
    xvT_v = xvT.rearrange("(c p) t -> p c t", p=128)

    with (
        tc.tile_pool(name="xin", bufs=2) as xin_p,
        tc.tile_pool(name="wgt", bufs=1) as w_p,
        tc.tile_pool(name="per", bufs=1) as per_p,
        tc.tile_pool(name="exp", bufs=3) as exp_p,
        tc.tile_pool(name="expt", bufs=3) as expt_p,
        tc.tile_pool(name="osb", bufs=3) as osb_p,
        tc.tile_pool(name="sm", bufs=6) as sm_p,
        tc.tile_pool(name="np_ps", bufs=2, space="PSUM") as np_ps,
        tc.tile_pool(name="st_ps", bufs=2, space="PSUM") as st_ps,
        tc.tile_pool(name="zt_ps", bufs=1, space="PSUM") as zt_ps,
    ):
        # ---- weights ----
        wq_sb = w_p.tile([128, DC, 128], F32, tag="wq")
        nc.sync.dma_start(out=wq_sb[:], in_=wqT.rearrange("(c p) j -> p c j", p=128))
        wk_sb = w_p.tile([128, DC, 128], F32, tag="wk")
        nc.sync.dma_start(out=wk_sb[:], in_=wkT.rearrange("(c p) j -> p c j", p=128))
        wv_sb = w_p.tile([128, DC, 128], F32, tag="wv")
        nc.sync.dma_start(out=wv_sb[:], in_=wvT.rearrange("(c p) j -> p c j", p=128))
        wo_sb = w_p.tile([64, NJ, D], F32, tag="wo")
        nc.sync.dma_start(
            out=wo_sb[:], in_=woT.rearrange("(j p) o -> p j o", p=64)
        )

        # ---- persistent tiles ----
        qT2 = per_p.tile([128, T], F32R, tag="qT2")  # [2 heads x 64 hd, t]
        kT2 = per_p.tile([128, T], F32R, tag="kT2")
        v2 = per_p.tile([128, T], F32R, tag="v2")    # [s%128, sblock*128 + e]
        zT2 = per_p.tile([64, NJ, T], F32, tag="zT2")  # [e, head, t] unnormalized
        recip = per_p.tile([128, NJ * TB], F32, tag="recip")

        # ---- projections ----
        for ts in range(NTS):
            tsl = slice(ts * TS, (ts + 1) * TS)
            xq_t = xin_p.tile([128, DC, TS], F32, tag="xin")
            nc.sync.dma_start(out=xq_t[:], in_=xqT_v[:, :, tsl])
            xk_t = xin_p.tile([128, DC, TS], F32, tag="xin")
            nc.sync.dma_start(out=xk_t[:], in_=xkT_v[:, :, tsl])
            xv_t = xin_p.tile([128, DC, TS], F32, tag="xin")
            nc.sync.dma_start(out=xv_t[:], in_=xvT_v[:, :, tsl])

            for w_sb, x_t, dst in ((wq_sb, xq_t, qT2), (wk_sb, xk_t, kT2)):
                ps = np_ps.tile([128, NPW], F32, tag="np")
                for d in range(DC):
                    for c in range(TS // 512):
                        nc.tensor.matmul(
                            ps[:, c * 512 : (c + 1) * 512],
                            w_sb[:, d, :],
                            x_t[:, d, c * 512 : (c + 1) * 512],
                            start=(d == 0),
                            stop=(d == DC - 1),
                        )
                nc.vector.tensor_copy(dst[:, tsl], ps[:])

            # v in natural layout: out[s, e] per 128-s-block
            # (one bank-owning accumulation group per tile: start= clears the
            # whole bank's has_written bits, so regions can't share a bank)
            for sb in range(TS // 128):
                ps = st_ps.tile([128, 512], F32, tag="st")
                for d in range(DC):
                    nc.tensor.matmul(
                        ps[:, 0:128],
                        xv_t[:, d, sb * 128 : (sb + 1) * 128],
                        wv_sb[:, d, :],
                        start=(d == 0),
                        stop=(d == DC - 1),
                    )
                base = ts * TS + sb * 128
                nc.vector.tensor_copy(v2[:, base : base + 128], ps[:, 0:128])

        # ---- normal pass: attn rows out ----
        for tb in range(TB):
            for j in range(NJ):
                jp = slice(64 * j, 64 * j + 64)
                et = exp_p.tile([128, T], F32, tag="exp")
                rs = sm_p.tile([128, NPC], F32, tag="rs")
                for c in range(NPC):
                    ps = np_ps.tile([128, NPW], F32, tag="np")
                    for cc in range(NPW // 512):
                        sl = slice(c * NPW + cc * 512, c * NPW + (cc + 1) * 512)
                        nc.tensor.matmul(
                            ps[:, cc * 512 : (cc + 1) * 512],
                            qT2[jp, tb * 128 : (tb + 1) * 128],
                            kT2[jp, sl],
                            start=True,
                            stop=True,
                        )
                    nc.scalar.activation(
                        et[:, c * NPW : (c + 1) * NPW],
                        ps[:],
                        AF.Exp,
                        scale=SCALE,
                        accum_out=rs[:, c : c + 1],
                    )
                rsum = sm_p.tile([128, 1], F32, tag="rsum")
                nc.vector.tensor_reduce(rsum[:], rs[:], axis=AX.X, op=ALU.add)
                ridx = j * TB + tb
                nc.vector.reciprocal(recip[:, ridx : ridx + 1], rsum[:])
                nc.vector.tensor_scalar_mul(et[:], et[:], recip[:, ridx : ridx + 1])
                nc.sync.dma_start(
                    out=attn2[j, tb * 128 : (tb + 1) * 128, :], in_=et[:]
                )

        # ---- transposed pass + attn^T v ----
        # each head's z^T accumulator owns its own PSUM bank for the whole
        # s-loop (start= clears has_written bank-wide)
        for q in range(NQ):
            tsl = slice(q * ZQW, (q + 1) * ZQW)
            zqs = [
                zt_ps.tile([64, ZQW], F32, tag=f"zt{j}", name=f"zq{j}")
                for j in range(NJ)
            ]
            for s in range(TB):
                for j in range(NJ):
                    jp = slice(64 * j, 64 * j + 64)
                    ps = st_ps.tile([128, ZQW], F32, tag="st")
                    nc.tensor.matmul(
                        ps[:, :],
                        kT2[jp, s * 128 : (s + 1) * 128],
                        qT2[jp, tsl],
                        start=True,
                        stop=True,
                    )
                    ex = expt_p.tile([128, ZQW], F32R, tag="expt")
                    nc.scalar.activation(ex[:], ps[:], AF.Exp, scale=SCALE)
                    nc.tensor.matmul(
                        zqs[j][:, :],
                        v2[:, s * 128 + 64 * j : s * 128 + 64 * j + 64],
                        ex[:],
                        start=(s == 0),
                        stop=(s == TB - 1),
                    )
            for j in range(NJ):
                nc.vector.tensor_copy(zT2[:, j, tsl], zqs[j][:, :])

        # ---- output projection (recip folded in per head) ----
        for tb in range(TB):
            po0 = st_ps.tile([128, 512], F32, tag="st")
            nc.tensor.matmul(
                po0[:, 0:D], zT2[:, 0, tb * 128 : (tb + 1) * 128], wo_sb[:, 0, :],
                start=True, stop=True,
            )
            po1 = st_ps.tile([128, 512], F32, tag="st")
            nc.tensor.matmul(
                po1[:, 0:D], zT2[:, 1, tb * 128 : (tb + 1) * 128], wo_sb[:, 1, :],
                start=True, stop=True,
            )
            tmp = osb_p.tile([128, D], F32, tag="tmp")
            nc.vector.tensor_scalar_mul(tmp[:], po0[:, 0:D], recip[:, tb : tb + 1])
            ot = osb_p.tile([128, D], F32, tag="ot")
            nc.vector.scalar_tensor_tensor(
                ot[:], po1[:, 0:D], recip[:, TB + tb : TB + tb + 1], tmp[:],
                op0=ALU.mult, op1=ALU.add,
            )
            nc.sync.dma_start(out=outp[tb * 128 : (tb + 1) * 128, :], in_=ot[:])


def _build_program(T=T):
    import concourse.mybir as mybir
    import concourse.tile as tile
    from concourse import bacc

    F32 = mybir.dt.float32
    nc = bacc.Bacc(
        "TRN2", target_bir_lowering=False, debug=False, num_devices=NCORES
    )
    ins = {
        "xqT": nc.dram_tensor("xqT", [D, T], F32, kind="ExternalInput").ap(),
        "xkT": nc.dram_tensor("xkT", [D, T], F32, kind="ExternalInput").ap(),
        "xvT": nc.dram_tensor("xvT", [D, T], F32, kind="ExternalInput").ap(),
        "wqT": nc.dram_tensor("wqT", [D, 128], F32, kind="ExternalInput").ap(),
        "wkT": nc.dram_tensor("wkT", [D, 128], F32, kind="ExternalInput").ap(),
        "wvT": nc.dram_tensor("wvT", [D, 128], F32, kind="ExternalInput").ap(),
        "woT": nc.dram_tensor("woT", [128, D], F32, kind="ExternalInput").ap(),
    }
    outs = {
        "attn2": nc.dram_tensor("attn2", [NJ, T, T], F32, kind="ExternalOutput").ap(),
        "outp": nc.dram_tensor("outp", [T, D], F32, kind="ExternalOutput").ap(),
    }
    with tile.TileContext(nc) as tc:
        build_mha(tc, outs, ins, T=T)
    nc.compile()
    return nc


_PROGRAM_CACHE = {}


def run_cores(in_maps, T=T):
    from concourse.bass_utils import run_bass_kernel_spmd

    if T not in _PROGRAM_CACHE:
        _PROGRAM_CACHE[T] = _build_program(T)
    nc = _PROGRAM_CACHE[T]
    return run_bass_kernel_spmd(nc, in_maps, list(range(NCORES))).results


def make_in_maps(queries, keys, values, Wq, Wk, Wv, Wo):
    f32 = np.float32
    in_maps = []
    for c in range(NCORES):
        b = c // 4
        h0 = 2 * (c % 4)
        rows = slice(h0 * HD, (h0 + 2) * HD)
        in_maps.append(
            {
                "xqT": np.ascontiguousarray(np.asarray(queries)[b].T, f32),
                "xkT": np.ascontiguousarray(np.asarray(keys)[b].T, f32),
                "xvT": np.ascontiguousarray(np.asarray(values)[b].T, f32),
                "wqT": np.ascontiguousarray(np.asarray(Wq)[rows].T, f32),
                "wkT": np.ascontiguousarray(np.asarray(Wk)[rows].T, f32),
                "wvT": np.ascontiguousarray(np.asarray(Wv)[rows].T, f32),
                "woT": np.ascontiguousarray(np.asarray(Wo)[:, rows].T, f32),
            }
        )
    return in_maps


def kernel(queries, keys, values, Wq, Wk, Wv, Wo, bo):
    in_maps = make_in_maps(queries, keys, values, Wq, Wk, Wv, Wo)
    res = run_cores(in_maps)
    bo = np.asarray(bo, np.float32)
    out = np.empty((B, T, D), np.float32)
    attn = np.empty((B, H, T, T), np.float32)
    for b in range(B):
        acc = None
        for c in range(4 * b, 4 * b + 4):
            h0 = 2 * (c % 4)
            attn[b, h0 : h0 + 2] = res[c]["attn2"]
            acc = res[c]["outp"] if acc is None else acc + res[c]["outp"]
        out[b] = acc + bo[None, :]
    return out, attn
